# revision 1
# baseline (speedup 1.0000x reference)
"""CompressedGPT2Attention on 8 TRN2 NeuronCores.

Sharding: core c = (batch b = c // 2, head-group g = c % 2) — data parallel on
B=4, tensor parallel over 16 heads (8 per group). Each core computes a partial
output [S, E] (its head-group's contribution, + output_bias on g==0 cores);
host sums the two partials per batch.

Per-core pipeline (matmul operands float32r = raw fp32 bits, PE rounds):
  hs_t [E, S] (host-transposed)
  v_aug    = hs @ Wv_aug  [S, 8*33] (per head 32 v-cols + ones col -> denominator)
  q^T, k^T = W^T @ hs_t   [512, S]  (2 heads per 128-partition tile; bias via
                                     ACT/DVE per-partition add on psum->sbuf copy)
  per head, scores^T[j, i] = k_h^T.T @ q_h^T (K=64); causal mask on diagonal
    blocks = extra PE accumulate of identity.T @ (-1e4 lower-triangle) so exp
    underflows to exactly 0
  exp via ScalarE (scale=1/8 folded), f32r out
  attn_aug^T[33, i] += v_aug_jc^T.T @ exp_jc  (PSUM accum over j-chunks)
  stage attn psum -> SBUF, normalize rows 0..31 by 1/row32 (DVE recip,
    gpsimd partition_broadcast, DVE mult)
  out[i, e] = attn^T.T @ Wout + bias (bias via gpsimd broadcast + DVE add)
"""

import numpy as np
from contextlib import ExitStack

import concourse.bass as bass
import concourse.bacc as bacc
import concourse.tile as tile
import concourse.mybir as mybir
from concourse.bass_utils import run_bass_kernel_spmd

F32 = mybir.dt.float32
F32R = mybir.dt.float32r
BF16 = mybir.dt.bfloat16
AF = mybir.ActivationFunctionType

B, S, E = 4, 2048, 1024
H, HD, R = 16, 64, 32
HG = 8                # heads per core
N_CORES = 8
SCALE = 1.0 / 8.0     # 1/sqrt(HD)
NEG = -1.0e4

_PROGRAM_CACHE = {}


def _build_program():
    nc = bacc.Bacc("TRN2", target_bir_lowering=False, debug=False,
                   num_devices=N_CORES)

    hs_t = nc.dram_tensor("hs_t", [E, S], F32R, kind="ExternalInput").ap()
    wq = nc.dram_tensor("wq", [E, 512], F32R, kind="ExternalInput").ap()
    wk = nc.dram_tensor("wk", [E, 512], F32R, kind="ExternalInput").ap()
    bqt = nc.dram_tensor("bqt", [128, 4], F32, kind="ExternalInput").ap()
    bkt = nc.dram_tensor("bkt", [128, 4], F32, kind="ExternalInput").ap()
    wv = nc.dram_tensor("wv", [E, HG * 33], F32R, kind="ExternalInput").ap()
    bv = nc.dram_tensor("bv", [1, HG * 33], F32, kind="ExternalInput").ap()
    wout = nc.dram_tensor("wout", [256, E], F32R, kind="ExternalInput").ap()
    bout = nc.dram_tensor("bout", [1, E], F32, kind="ExternalInput").ap()
    bout_r = nc.dram_tensor("bout_r", [1, E], F32R, kind="ExternalInput").ap()
    tri = nc.dram_tensor("tri", [128, 128], F32, kind="ExternalInput").ap()
    out = nc.dram_tensor("out", [S, E], F32, kind="ExternalOutput").ap()

    with tile.TileContext(nc) as tc, ExitStack() as ctx:
        persist = ctx.enter_context(tc.tile_pool(name="persist", bufs=1))
        big = ctx.enter_context(tc.tile_pool(name="big", bufs=1))

        # ---- persistent activations / constants ----
        q_sb = [persist.tile([128, S], F32R, name=f"q{m}", tag=f"q{m}") for m in range(4)]
        k_sb = [persist.tile([128, S], F32R, name=f"k{m}", tag=f"k{m}") for m in range(4)]
        v_sb = [persist.tile([128, HG * 33], F32R, name=f"v{sc}", tag=f"v{sc}")
                for sc in range(16)]

        tri_sb = persist.tile([128, 128], F32, name="tri", tag="tri")
        wout_sb = [persist.tile([128, E], F32R, name=f"wo{t}", tag=f"wo{t}")
                   for t in range(2)]
        ones_f = persist.tile([1, 128], F32, name="ones_f", tag="ones_f")
        nc.vector.memset(ones_f, 1.0)
        ones_r = persist.tile([1, 128], F32R, name="ones_r", tag="ones_r")
        nc.vector.tensor_copy(out=ones_r, in_=ones_f)

        # =========== phase 1: projections ===========
        with ExitStack() as pctx:
            w_pool = pctx.enter_context(tc.tile_pool(name="w", bufs=1))
            b_pool = pctx.enter_context(tc.tile_pool(name="b", bufs=1))
            pp = pctx.enter_context(tc.tile_pool(name="pp", bufs=4, space="PSUM"))

            # DMA order: small weights for the first groups, then hs chunks,
            # then the rest. Dynamic HW queues round-robin so these overlap.
            wv_sb = []
            for ec in range(8):
                t = w_pool.tile([128, HG * 33], F32R, name=f"wv{ec}", tag=f"wv{ec}")
                nc.sync.dma_start(out=t, in_=wv[ec * 128:(ec + 1) * 128, :])
                wv_sb.append(t)
            bv_sb = b_pool.tile([1, HG * 33], F32, name="bv", tag="bv")
            nc.sync.dma_start(out=bv_sb, in_=bv)
            bv_bc = b_pool.tile([128, HG * 33], F32, name="bv_bc", tag="bv_bc")
            nc.gpsimd.partition_broadcast(bv_bc, bv_sb)
            nc.sync.dma_start(out=tri_sb, in_=tri)
            for t in range(2):
                nc.sync.dma_start(out=wout_sb[t], in_=wout[t * 128:(t + 1) * 128, :])

            hs_pool = pctx.enter_context(tc.tile_pool(name="hsp", bufs=1))
            hs_sb = []
            for ec in range(8):
                pool_for = big if ec < 2 else hs_pool
                t = pool_for.tile([128, S], F32R, name=f"hs{ec}", tag=f"hs{ec}")
                hs_sb.append(t)
            for half in range(2):
                hsl = slice(half * 1024, half * 1024 + 1024)
                for ec in range(8):
                    nc.sync.dma_start(out=hs_sb[ec][:, hsl],
                                      in_=hs_t[ec * 128:(ec + 1) * 128, hsl])

            bqt_sb = b_pool.tile([128, 4], F32, name="bqt", tag="bqt")
            nc.sync.dma_start(out=bqt_sb, in_=bqt)
            bkt_sb = b_pool.tile([128, 4], F32, name="bkt", tag="bkt")
            nc.sync.dma_start(out=bkt_sb, in_=bkt)

            # v_aug first (small, unblocks attention earliest):
            # psum[s 128, 264] = sum_e hs_t[e, s].T @ wv[e, :]
            for sc in range(16):
                ps = pp.tile([128, HG * 33], F32, name="vproj", tag="vproj")
                for ec in range(8):
                    nc.tensor.matmul(ps, hs_sb[ec][:, sc * 128:(sc + 1) * 128],
                                     wv_sb[ec], start=(ec == 0), stop=(ec == 7))
                nc.vector.tensor_add(out=v_sb[sc], in0=ps, in1=bv_bc)

            # q^T / k^T: psum[col 128, s 512] = sum_e w[e, col].T @ hs_t[e, s]
            # (weight slots shared between q and k to fit SBUF)
            for which, w_dram, bias_sb, dst in ((0, wq, bqt_sb, q_sb),
                                                (1, wk, bkt_sb, k_sb)):
                w_cur = []
                for ec in range(8):
                    t = w_pool.tile([128, 512], F32R, name=f"w{which}_{ec}",
                                    tag=f"w{which}_{ec}")
                    nc.sync.dma_start(out=t, in_=w_dram[ec * 128:(ec + 1) * 128, :])
                    w_cur.append(t)
                for m in range(4):
                    for nb in range(4):
                        ps = pp.tile([128, 512], F32, name="proj", tag="proj")
                        sl = slice(nb * 512, nb * 512 + 512)
                        for ec in range(8):
                            nc.tensor.matmul(ps, w_cur[ec][:, m * 128:(m + 1) * 128],
                                             hs_sb[ec][:, sl],
                                             start=(ec == 0), stop=(ec == 7))
                        # psum->sbuf with per-partition bias add; alternate
                        # ACT/DVE to balance engines
                        if nb % 2 == 0:
                            nc.scalar.activation(out=dst[m][:, sl], in_=ps,
                                                 func=AF.Identity,
                                                 bias=bias_sb[:, m:m + 1], scale=1.0)
                        else:
                            nc.vector.tensor_scalar_add(out=dst[m][:, sl], in0=ps,
                                                        scalar1=bias_sb[:, m:m + 1])

        # =========== phase 2: attention (ib2-major) + overlapped outproj ===========
        with ExitStack() as actx:
            sc_pool = actx.enter_context(tc.tile_pool(name="scps", bufs=1, space="PSUM"))
            at_pool = actx.enter_context(tc.tile_pool(name="atps", bufs=1, space="PSUM"))
            exp_pool = actx.enter_context(tc.tile_pool(name="exp", bufs=4))
            nrm_pool = actx.enter_context(tc.tile_pool(name="nrm", bufs=2))
            ob_pool = actx.enter_context(tc.tile_pool(name="ob", bufs=4))

            bout_r_sb = ob_pool.tile([1, E], F32R, name="bor", tag="bor", bufs=1)
            nc.sync.dma_start(out=bout_r_sb, in_=bout_r)
            bo_f_sb = ob_pool.tile([1, E], F32, name="bo_f", tag="bo_f", bufs=1)
            nc.sync.dma_start(out=bo_f_sb, in_=bout)
            bout_bc = ob_pool.tile([128, E], F32, name="bo_bc", tag="bo_bc", bufs=1)
            nc.gpsimd.partition_broadcast(bout_bc, bo_f_sb)
            attn_sb = [big.tile([128, S], F32R, name=f"attn{t}", tag=f"hs{t}")
                       for t in range(2)]

            def outproj(it, eb, psum_tag):
                """out[it*128:+128, eb*512:+512] = attn^T.T @ wout + bias."""
                sl = slice(eb * 512, eb * 512 + 512)
                ps = sc_pool.tile([128, 512], F32, name="ops", tag="s", bufs=2)
                use_act = (it + eb) % 2 == 0
                for t in range(2):
                    nc.tensor.matmul(ps, attn_sb[t][:, it * 128:(it + 1) * 128],
                                     wout_sb[t][:, sl],
                                     start=(t == 0), stop=(t == 1 and not use_act))
                if use_act:
                    nc.tensor.matmul(ps, ones_r, bout_r_sb[:, sl],
                                     start=False, stop=True)
                    ot = ob_pool.tile([128, 512], F32, name="ot", tag="ot")
                    nc.scalar.activation(out=ot, in_=ps, func=AF.Copy,
                                         bias=0.0, scale=1.0)
                else:
                    ot = ob_pool.tile([128, 512], F32, name="ot", tag="ot")
                    nc.vector.tensor_add(out=ot, in0=ps, in1=bout_bc[:, sl])
                nc.sync.dma_start(out=out[it * 128:(it + 1) * 128, sl], in_=ot)

            for ib2 in range(2):
                ibase = ib2 * 1024
                jcs = list(range(8 * (ib2 + 1)))
                ilo = {jc: max(jc * 128 - ibase, 0) for jc in jcs}
                bank_jcs = {nb: [jc for jc in jcs if ilo[jc] < nb * 512 + 512]
                            for nb in range(2)}

                for h in range(8):
                    pairm = h // 2
                    dpart = slice((h % 2) * 64, (h % 2) * 64 + 64)
                    attn_ps = at_pool.tile([33, 1024], F32, name="attn_ps",
                                           tag="a", bufs=2)

                    for jc in jcs:
                        lo = ilo[jc]
                        diag = jc >= 8 * ib2
                        sp = sc_pool.tile([128, 1024], F32, name="s", tag="s", bufs=2)
                        for nb in range(2):
                            a = max(lo, nb * 512)
                            bb = nb * 512 + 512
                            if a >= bb:
                                continue
                            nc.tensor.matmul(
                                sp[:, a:bb],
                                k_sb[pairm][dpart, jc * 128:(jc + 1) * 128],
                                q_sb[pairm][dpart, ibase + a:ibase + bb],
                                start=True, stop=True)
                        et = exp_pool.tile([128, 1024], F32R, name="e", tag="e", bufs=4)
                        nc.scalar.activation(out=et[:, lo:1024],
                                             in_=sp[:, lo:1024],
                                             func=AF.Exp, scale=SCALE)
                        if diag:
                            nc.vector.tensor_mul(
                                out=et[:, lo:lo + 128],
                                in0=et[:, lo:lo + 128].bitcast(F32),
                                in1=tri_sb)
                        for nb in range(2):
                            a = max(lo, nb * 512)
                            bb = nb * 512 + 512
                            if a >= bb:
                                continue
                            nc.tensor.matmul(
                                attn_ps[:, a:bb],
                                v_sb[jc][:, h * 33:(h + 1) * 33],
                                et[:, a:bb],
                                start=(jc == bank_jcs[nb][0]),
                                stop=(jc == bank_jcs[nb][-1]))

                    # stage to SBUF (frees psum), then normalize rows /= row 32
                    t, roff = h // 4, (h % 4) * 32
                    araw = nrm_pool.tile([33, 1024], F32, name="araw", tag="araw")
                    nc.vector.tensor_copy(out=araw, in_=attn_ps)
                    rec = nrm_pool.tile([1, 1024], F32, name="rec", tag="rec")
                    nc.vector.reciprocal(out=rec, in_=araw[32:33, :])
                    rec_bc = nrm_pool.tile([32, 1024], F32, name="recbc", tag="recbc")
                    nc.gpsimd.partition_broadcast(rec_bc, rec)
                    nc.vector.tensor_mul(
                        out=attn_sb[t][roff:roff + 32, ibase:ibase + 1024],
                        in0=araw[0:32, :],
                        in1=rec_bc)

            # outproj phase (psum slots freed by the attention pools rotate in)
            for it in range(16):
                for eb in range(2):
                    outproj(it, eb, psum_tag="s")

    nc.compile()
    return nc


def _get_program():
    if "nc" not in _PROGRAM_CACHE:
        _PROGRAM_CACHE["nc"] = _build_program()
    return _PROGRAM_CACHE["nc"]


def kernel(hidden_states, q_weight, q_bias, k_weight, k_bias,
           low_rank_value_weight, low_rank_value_bias,
           low_rank_output_weight, output_bias):
    hidden_states = np.asarray(hidden_states, dtype=np.float32)
    q_weight = np.asarray(q_weight, dtype=np.float32)
    q_bias = np.asarray(q_bias, dtype=np.float32)
    k_weight = np.asarray(k_weight, dtype=np.float32)
    k_bias = np.asarray(k_bias, dtype=np.float32)
    wv_full = np.asarray(low_rank_value_weight, dtype=np.float32)
    bv_full = np.asarray(low_rank_value_bias, dtype=np.float32)
    wout_full = np.asarray(low_rank_output_weight, dtype=np.float32)
    output_bias = np.asarray(output_bias, dtype=np.float32)

    tri = np.triu(np.ones((128, 128), dtype=np.float32))  # keep where j <= i

    in_maps = []
    for c in range(N_CORES):
        b, g = c // 2, c % 2
        hs_t = np.ascontiguousarray(hidden_states[b].T)          # [E, S]
        cols = slice(g * 512, (g + 1) * 512)                     # q/k head cols
        vcols = slice(g * 256, (g + 1) * 256)                    # v head cols
        wv_aug = np.zeros((E, HG * 33), dtype=np.float32)
        bv_aug = np.zeros((1, HG * 33), dtype=np.float32)
        wv_g = wv_full[:, vcols].reshape(E, HG, R)
        bv_g = bv_full[vcols].reshape(HG, R)
        for h in range(HG):
            wv_aug[:, h * 33:h * 33 + 32] = wv_g[:, h, :]
            bv_aug[0, h * 33:h * 33 + 32] = bv_g[h]
            bv_aug[0, h * 33 + 32] = 1.0
        in_maps.append({
            "hs_t": hs_t,
            "wq": np.ascontiguousarray(q_weight[:, cols]),
            "wk": np.ascontiguousarray(k_weight[:, cols]),
            "bqt": np.ascontiguousarray(q_bias[cols].reshape(4, 128).T),
            "bkt": np.ascontiguousarray(k_bias[cols].reshape(4, 128).T),
            "wv": wv_aug,
            "bv": bv_aug,
            "wout": np.ascontiguousarray(wout_full[vcols, :]),
            "bout": (output_bias if g == 0
                     else np.zeros_like(output_bias))[None, :],
            "bout_r": (output_bias if g == 0
                       else np.zeros_like(output_bias))[None, :],
            "tri": tri,
        })

    nc = _get_program()
    res = run_bass_kernel_spmd(nc, in_maps, list(range(N_CORES)))
    out = np.empty((B, S, E), dtype=np.float32)
    for b in range(B):
        out[b] = res.results[2 * b]["out"] + res.results[2 * b + 1]["out"]
    return out

